# revision 13
# baseline (speedup 1.0000x reference)
"""Distributed sliding-window GQA attention kernel for 8 Trainium2 NeuronCores.

Problem (full shapes): x [1, 2048, 4096] f32, wq [4096, 4096], wk/wv [4096, 1024],
wo [4096, 4096], rotary freqs [2048, 64]. 32 q heads / 8 kv heads (GQA group 4),
head_dim 128, causal sliding window 1024.

Sharding (tensor parallel over heads): core c owns q heads 4c..4c+3 and kv head c
(wq/wk/wv column shards). The output projection is column-sharded: each core
AllGathers the (bf16) attention outputs of all heads per 512-token chunk and
computes out[:, 512c:512c+512] with its wo column shard. Host concatenates.

All matmuls run in bf16 with f32 PSUM accumulation. Layout choices:
 - x is pre-transposed on host to x_T [4096, 2048]; per-core QKV computes
   q/k transposed [head_dim, seq] via lhsT=weight tile, rhs=x_T.
 - head_dim is pre-permuted (even indices then odd) in wq/wk columns so RoPE
   operates on contiguous partition halves [0:64], [64:128].
 - scores are computed transposed S_T[j, i], exp'd on the scalar engine
   (1/sqrt(128) folded into the activation scale), masked multiplicatively,
   and fed straight into PV with v tiles [j, d] as stationary.
 - softmax denominator: ones-vector matmul accumulated alongside PV;
   normalization = fast-approx reciprocal + gpsimd partition broadcast.
 - sliding window: per 512-query chunk only j-blocks in [i0-1024, i0+512),
   boundary blocks narrowed to their non-zero column span.
DMAs are batched (multiple 128-row tiles per transfer via 3D access patterns)
and split across the sync and gpsimd queues to keep descriptor issue off the
critical path.
"""

import math
from contextlib import ExitStack

import numpy as np

import concourse.bass as bass
import concourse.mybir as mybir
import concourse.tile as tile
from concourse import bacc
from concourse.bass_utils import run_bass_kernel_spmd
from concourse.masks import make_identity

# ---- problem constants (hardcoded; kernel.py must be self-contained) ----
B = 1
S = 2048
D = 4096
N_Q_HEADS = 32
HD = 128
WINDOW = 1024
N_CORES = 8

QH = N_Q_HEADS // N_CORES  # 4 local q heads
P = 128
CH = 512  # seq chunk
NCH = S // CH  # 4
DT = D // P  # 32 contraction tiles
ST = CH // P  # 4 s-tiles per chunk
DC = D // N_CORES  # 512 output columns per core

F32 = mybir.dt.float32
BF16 = mybir.dt.bfloat16

_BUILT = None


def _span(rel):
    """Non-zero column span (c0, c1) and mask slot for a j-block at relative
    position rel = (j0 - (i0 - WINDOW)) // 128 in 0..11."""
    if rel <= 3:
        return 0, 128 * (rel + 1), rel  # window-edge wedge
    if rel <= 7:
        return 0, CH, None  # fully inside window
    return 128 * (rel - 8), CH, 4 + (rel - 8)  # causal wedge


def _build():
    nc = bacc.Bacc("TRN2", target_bir_lowering=False, debug=False, num_devices=N_CORES)

    xT_ext = nc.declare_dram_parameter("xT", [D, S], F32, isOutput=False)
    wq_ext = nc.declare_dram_parameter("wq", [D, QH * HD], F32, isOutput=False)
    wk_ext = nc.declare_dram_parameter("wk", [D, HD], F32, isOutput=False)
    wv_ext = nc.declare_dram_parameter("wv", [D, HD], F32, isOutput=False)
    wo_ext = nc.declare_dram_parameter("wo", [D, DC], F32, isOutput=False)
    cos_ext = nc.declare_dram_parameter("cosT", [HD // 2, S], F32, isOutput=False)
    sin_ext = nc.declare_dram_parameter("sinT", [HD // 2, S], F32, isOutput=False)
    mask_ext = nc.declare_dram_parameter("masks", [8, P, CH], F32, isOutput=False)
    out_ext = nc.declare_dram_parameter("out", [S, DC], F32, isOutput=True)

    inv_sqrt_hd = 1.0 / math.sqrt(HD)

    with tile.TileContext(nc) as tc:
        with ExitStack() as stack:
            pool = lambda *a, **kw: stack.enter_context(tc.tile_pool(*a, **kw))
            wq_pool = pool(name="wq", bufs=DT // 2)  # 16 x [128, 2, 512]
            wk_pool = pool(name="wk", bufs=DT // 8)  # 4 x [128, 8, 128]
            wv_pool = pool(name="wv", bufs=DT // 8)
            x_pool = pool(name="xbf", bufs=20)  # [128, 2, 512] pairs
            k_pool = pool(name="kt", bufs=NCH)
            v_pool = pool(name="vt", bufs=NCH * ST)
            q_pool = pool(name="qt", bufs=2 * QH)
            att_pool = pool(name="att", bufs=2 * QH)
            mask_pool = pool(name="mask", bufs=8)
            e_pool = pool(name="et", bufs=3)
            r_pool = pool(name="rtmp", bufs=4)
            xst_pool = pool(name="xst", bufs=3)  # f32 staging [128, 2, 512]
            vts_pool = pool(name="vts", bufs=2)
            rb_pool = pool(name="rb", bufs=1)
            rc_pool = pool(name="rc", bufs=2)
            at_pool = pool(name="atst", bufs=2)  # [128, 32, 128] gathered att
            wot_pool = pool(name="wot", bufs=2)  # [128, 4, 512] wo stream
            out_pool = pool(name="osb", bufs=2)
            misc_pool = pool(name="misc", bufs=1)
            acc_ps = pool(name="accps", bufs=3, space="PSUM")
            qkv_ps = acc_ps
            s_ps = acc_ps
            pv_ps = pool(name="pvps", bufs=2, space="PSUM")
            rs_ps = pool(name="rsps", bufs=1, space="PSUM")
            op_ps = pool(name="opps", bufs=2, space="PSUM")
            dram_pool = pool(name="dram", bufs=1, space="DRAM")
            ccin_pool = pool(name="ccin", bufs=4, space="DRAM")
            gath_pool = pool(name="gath", bufs=4, space="DRAM")

            # ---- small constants (needed by chunk 0) ----
            cos_sb = misc_pool.tile([HD // 2, S], F32, tag="cos")
            nc.sync.dma_start(out=cos_sb[:], in_=cos_ext[:])
            sin_sb = misc_pool.tile([HD // 2, S], F32, tag="sin")
            nc.sync.dma_start(out=sin_sb[:], in_=sin_ext[:])
            ident = misc_pool.tile([P, P], BF16, tag="ident")
            make_identity(nc, ident[:])
            ones_bf = misc_pool.tile([P, 1], BF16, tag="ones")
            nc.vector.memset(ones_bf[:], 1.0)
            mask_sb = []
            for r in range(8):
                mst = xst_pool.tile([P, CH], F32, tag="xst", name=f"mst{r}")
                nc.sync.dma_start(out=mst[:], in_=mask_ext[r])
                mb = mask_pool.tile([P, CH], BF16, tag="mask", name=f"mask{r}")
                nc.scalar.copy(mb[:], mst[:])
                mask_sb.append(mb)

            # tile handles
            wq_t = [None] * (DT // 2)  # [128, 2, 512] bf16
            wk_t = [None] * (DT // 8)  # [128, 8, 128] bf16
            wv_t = [None] * (DT // 8)
            x_tiles = {}  # (I, g) -> [128, 2, 512] bf16
            k_chunks = [None] * NCH
            v_tiles = [None] * (NCH * ST)
            q_tiles = {}
            att_tiles = {}
            gath = [None] * NCH
            wo_dram = dram_pool.tile([D, DC], BF16, tag="wod")

            def wq_ap(Dt):
                return wq_t[Dt // 2][:, Dt % 2, :]

            def x_ap(I, Dt):
                return x_tiles[(I, Dt // 2)][:, Dt % 2, :]

            def emit_x_group(I, g):
                """Load x_T rows [256g, 256(g+1)) cols of chunk I, convert."""
                st_ = xst_pool.tile([P, 2, CH], F32, tag="xst", name=f"xst{I}_{g}")
                nc.sync.dma_start(
                    out=st_[:],
                    in_=xT_ext[
                        g * 2 * P : (g + 1) * 2 * P, I * CH : (I + 1) * CH
                    ].rearrange("(po pi) s -> pi po s", pi=P),
                )
                xb = x_pool.tile([P, 2, CH], BF16, tag="xbf", name=f"xbf{I}_{g}")
                nc.scalar.copy(xb[:], st_[:])
                x_tiles[(I, g)] = xb

            def emit_x_chunk(I):
                for g in range(DT // 2):
                    emit_x_group(I, g)

            def rope(ps, out_bf, I):
                cs = cos_sb[:, I * CH : (I + 1) * CH]
                sn = sin_sb[:, I * CH : (I + 1) * CH]
                m1 = r_pool.tile([HD // 2, CH], F32, tag="m", name="m1")
                nc.vector.tensor_mul(m1[:], ps[0:64, :], cs)
                m2 = r_pool.tile([HD // 2, CH], F32, tag="m", name="m2")
                nc.vector.tensor_mul(m2[:], ps[64:128, :], sn)
                nc.vector.tensor_sub(out_bf[0:64, :], m1[:], m2[:])
                m3 = r_pool.tile([HD // 2, CH], F32, tag="m", name="m3")
                nc.vector.tensor_mul(m3[:], ps[0:64, :], sn)
                m4 = r_pool.tile([HD // 2, CH], F32, tag="m", name="m4")
                nc.vector.tensor_mul(m4[:], ps[64:128, :], cs)
                nc.vector.tensor_add(out_bf[64:128, :], m3[:], m4[:])

            def emit_qkv(I):
                for h in range(QH):
                    ps = qkv_ps.tile([P, CH], F32, tag="acc", name=f"psq{I}_{h}")
                    for Dt in range(DT):
                        nc.tensor.matmul(
                            ps[:],
                            wq_ap(Dt)[:, h * HD : (h + 1) * HD],
                            x_ap(I, Dt),
                            start=(Dt == 0),
                            stop=(Dt == DT - 1),
                        )
                    qb = q_pool.tile([P, CH], BF16, tag="qb", name=f"qb{I}_{h}")
                    rope(ps, qb, I)
                    q_tiles[(I, h)] = qb
                psk = qkv_ps.tile([P, CH], F32, tag="acc", name=f"psk{I}")
                for Dt in range(DT):
                    nc.tensor.matmul(
                        psk[:],
                        wk_t[Dt // 8][:, Dt % 8, :],
                        x_ap(I, Dt),
                        start=(Dt == 0),
                        stop=(Dt == DT - 1),
                    )
                kb = k_pool.tile([P, CH], BF16, tag="kb", name=f"kb{I}")
                rope(psk, kb, I)
                k_chunks[I] = kb
                psv = qkv_ps.tile([P, CH], F32, tag="acc", name=f"psv{I}")
                for Dt in range(DT):
                    nc.tensor.matmul(
                        psv[:],
                        wv_t[Dt // 8][:, Dt % 8, :],
                        x_ap(I, Dt),
                        start=(Dt == 0),
                        stop=(Dt == DT - 1),
                    )
                vT = vts_pool.tile([P, CH], BF16, tag="vT", name=f"vT{I}")
                nc.vector.tensor_copy(vT[:], psv[:])
                for sb in range(ST):
                    trp = qkv_ps.tile([P, P], BF16, tag="acc", name=f"trp{I}_{sb}")
                    nc.tensor.transpose(trp[:], vT[:, sb * P : (sb + 1) * P], ident[:])
                    vb = v_pool.tile([P, P], BF16, tag="vb", name=f"vb{I}_{sb}")
                    nc.vector.tensor_copy(vb[:], trp[:])
                    v_tiles[I * ST + sb] = vb

            def emit_attn(I):
                i0 = I * CH
                jlo = max(0, i0 - WINDOW)
                n_j = (i0 + CH - jlo) // P
                for h in range(QH):
                    pv = pv_ps.tile([P, CH], F32, tag="pv", name=f"pv{I}_{h}")
                    rs = rs_ps.tile([1, CH], F32, tag="rs", name=f"rs{I}_{h}")
                    qb = q_tiles[(I, h)]
                    for idx in range(n_j):
                        j0 = jlo + idx * P
                        rel = (j0 - (i0 - WINDOW)) // P
                        c0, c1, slot = _span(rel)
                        kb = k_chunks[j0 // CH]
                        koff = j0 % CH
                        sps = s_ps.tile([P, CH], F32, tag="acc", name=f"sps{I}_{h}_{idx}")
                        nc.tensor.matmul(
                            sps[:, c0:c1],
                            kb[:, koff : koff + P],
                            qb[:, c0:c1],
                            start=True,
                            stop=True,
                            skip_group_check=True,
                        )
                        et = e_pool.tile([P, CH], BF16, tag="et", name=f"et{I}_{h}_{idx}")
                        nc.scalar.activation(
                            et[:, c0:c1],
                            sps[:, c0:c1],
                            mybir.ActivationFunctionType.Exp,
                            scale=inv_sqrt_hd,
                        )
                        if slot is not None:
                            nc.vector.tensor_mul(
                                et[:, c0:c1], et[:, c0:c1], mask_sb[slot][:, c0:c1]
                            )
                        nc.tensor.matmul(
                            pv[:, c0:c1],
                            v_tiles[j0 // P][:],
                            et[:, c0:c1],
                            start=(idx == 0),
                            stop=(idx == n_j - 1),
                            skip_group_check=True,
                        )
                        nc.tensor.matmul(
                            rs[:, c0:c1],
                            ones_bf[:],
                            et[:, c0:c1],
                            start=(idx == 0),
                            stop=(idx == n_j - 1),
                            skip_group_check=True,
                        )
                    rc = rc_pool.tile([1, CH], F32, tag="rc", name=f"rc{I}_{h}")
                    nc.vector.reciprocal(rc[:], rs[:])
                    rb = rb_pool.tile([P, CH], F32, tag="rb", name=f"rb{I}_{h}")
                    nc.gpsimd.partition_broadcast(rb[:], rc[:])
                    ab = att_pool.tile([P, CH], BF16, tag="ab", name=f"ab{I}_{h}")
                    nc.vector.tensor_mul(ab[:], pv[:], rb[:])
                    att_tiles[(I, h)] = ab

            def emit_ship(I):
                ci = ccin_pool.tile([QH * HD, CH], BF16, tag="ci", name=f"ci{I}")
                for h in range(QH):
                    nc.gpsimd.dma_start(
                        out=ci[h * HD : (h + 1) * HD, :], in_=att_tiles[(I, h)][:]
                    )
                go = gath_pool.tile([D, CH], BF16, addr_space="Shared", tag="go", name=f"go{I}")
                nc.gpsimd.collective_compute(
                    "AllGather",
                    mybir.AluOpType.bypass,
                    replica_groups=[list(range(N_CORES))],
                    ins=[ci[:].opt()],
                    outs=[go[:].opt()],
                )
                gath[I] = go

            def emit_outproj(I):
                for g in range(ST // 2):
                    ats = []
                    for k in range(2):
                        st_idx = g * 2 + k
                        at = at_pool.tile([P, DT, P], BF16, tag="at", name=f"at{I}_{st_idx}")
                        nc.gpsimd.dma_start(
                            out=at[:],
                            in_=gath[I][:, st_idx * P : (st_idx + 1) * P].rearrange(
                                "(po pi) c -> pi po c", pi=P
                            ),
                        )
                        ats.append(at)
                    pso = [
                        op_ps.tile([P, CH], F32, tag="op", name=f"pso{I}_{g}_{k}")
                        for k in range(2)
                    ]
                    for Dtg in range(DT // 4):
                        wt = wot_pool.tile([P, 4, CH], BF16, tag="wt", name=f"wt{I}_{g}_{Dtg}")
                        nc.gpsimd.dma_start(
                            out=wt[:],
                            in_=wo_dram[Dtg * 4 * P : (Dtg + 1) * 4 * P, :].rearrange(
                                "(po pi) e -> pi po e", pi=P
                            ),
                        )
                        for d4 in range(4):
                            Dt = Dtg * 4 + d4
                            for k in range(2):
                                nc.tensor.matmul(
                                    pso[k][:],
                                    ats[k][:, Dt, :],
                                    wt[:, d4, :],
                                    start=(Dt == 0),
                                    stop=(Dt == DT - 1),
                                )
                    for k in range(2):
                        st_idx = g * 2 + k
                        ob = out_pool.tile([P, CH], F32, tag="ob", name=f"ob{I}_{st_idx}")
                        nc.vector.tensor_copy(ob[:], pso[k][:])
                        nc.sync.dma_start(
                            out=out_ext[
                                I * CH + st_idx * P : I * CH + (st_idx + 1) * P, :
                            ],
                            in_=ob[:],
                        )

            def emit_wo_setup():
                for g in range(DT // 2):
                    st_ = xst_pool.tile([P, 2, CH], F32, tag="xst", name=f"wost{g}")
                    nc.gpsimd.dma_start(
                        out=st_[:],
                        in_=wo_ext[g * 2 * P : (g + 1) * 2 * P, :].rearrange(
                            "(po pi) e -> pi po e", pi=P
                        ),
                    )
                    wb = wot_pool.tile(
                        [P, 2, CH], BF16, tag="wt", name=f"wob{g}"
                    )
                    nc.vector.tensor_copy(wb[:], st_[:])
                    nc.gpsimd.dma_start(
                        out=wo_dram[g * 2 * P : (g + 1) * 2 * P, :].rearrange(
                            "(po pi) e -> pi po e", pi=P
                        ),
                        in_=wb[:],
                    )

            # ---- emission schedule ----
            # chunk 0: interleave x groups with weight groups so QKV starts early
            for g in range(DT // 2):
                emit_x_group(0, g)
                st_ = xst_pool.tile([P, 2, CH], F32, tag="xst", name=f"wqst{g}")
                nc.scalar.dma_start(
                    out=st_[:],
                    in_=wq_ext[g * 2 * P : (g + 1) * 2 * P, :].rearrange(
                        "(po pi) c -> pi po c", pi=P
                    ),
                )
                wqb = wq_pool.tile([P, 2, QH * HD], BF16, tag="wqb", name=f"wqb{g}")
                nc.scalar.copy(wqb[:], st_[:])
                wq_t[g] = wqb
                if g % 4 == 0:
                    gg = g // 4
                    stk = xst_pool.tile([P, 8, HD], F32, tag="xst", name=f"wkst{gg}")
                    nc.gpsimd.dma_start(
                        out=stk[:],
                        in_=wk_ext[gg * 8 * P : (gg + 1) * 8 * P, :].rearrange(
                            "(po pi) c -> pi po c", pi=P
                        ),
                    )
                    wkb = wk_pool.tile([P, 8, HD], BF16, tag="wkb", name=f"wkb{gg}")
                    nc.scalar.copy(wkb[:], stk[:])
                    wk_t[gg] = wkb
                    stv = xst_pool.tile([P, 8, HD], F32, tag="xst", name=f"wvst{gg}")
                    nc.gpsimd.dma_start(
                        out=stv[:],
                        in_=wv_ext[gg * 8 * P : (gg + 1) * 8 * P, :].rearrange(
                            "(po pi) c -> pi po c", pi=P
                        ),
                    )
                    wvb = wv_pool.tile([P, 8, HD], BF16, tag="wvb", name=f"wvb{gg}")
                    nc.scalar.copy(wvb[:], stv[:])
                    wv_t[gg] = wvb

            emit_qkv(0)
            emit_x_chunk(1)
            emit_wo_setup()
            emit_attn(0)
            emit_ship(0)

            emit_qkv(1)
            emit_x_chunk(2)
            emit_outproj(0)
            emit_attn(1)
            emit_ship(1)

            emit_qkv(2)
            emit_x_chunk(3)
            emit_outproj(1)
            emit_attn(2)
            emit_ship(2)

            emit_qkv(3)
            emit_outproj(2)
            emit_attn(3)
            emit_ship(3)

            emit_outproj(3)

    nc.compile()
    return nc


def _prep_inputs(x, freqs_cos, freqs_sin, wq, wk, wv, wo):
    """Shard + lay out the full inputs for the 8 cores."""
    xT = np.ascontiguousarray(x.reshape(S, D).T).astype(np.float32)
    cosT = np.ascontiguousarray(freqs_cos.T).astype(np.float32)
    sinT = np.ascontiguousarray(freqs_sin.T).astype(np.float32)

    perm = np.concatenate([np.arange(0, HD, 2), np.arange(1, HD, 2)])

    jj = np.arange(P)[:, None]
    ii = np.arange(CH)[None, :]
    masks = np.zeros((8, P, CH), np.float32)
    for r in range(4):
        masks[r] = (ii - jj <= 128 * r).astype(np.float32)
    for r in range(8, 12):
        masks[4 + r - 8] = (ii - jj >= 128 * (r - 8)).astype(np.float32)

    in_maps = []
    for c in range(N_CORES):
        q_cols = np.concatenate([(QH * c + h) * HD + perm for h in range(QH)])
        k_cols = c * HD + perm
        in_maps.append(
            {
                "xT": xT,
                "wq": np.ascontiguousarray(wq[:, q_cols]).astype(np.float32),
                "wk": np.ascontiguousarray(wk[:, k_cols]).astype(np.float32),
                "wv": np.ascontiguousarray(wv[:, c * HD : (c + 1) * HD]).astype(
                    np.float32
                ),
                "wo": np.ascontiguousarray(wo[:, c * DC : (c + 1) * DC]).astype(
                    np.float32
                ),
                "cosT": cosT,
                "sinT": sinT,
                "masks": masks,
            }
        )
    return in_maps


def kernel(x, freqs_cos, freqs_sin, wq, wk, wv, wo, _trace=False, _result_box=None):
    global _BUILT
    x = np.asarray(x, dtype=np.float32)
    if _BUILT is None:
        _BUILT = _build()
    nc = _BUILT
    in_maps = _prep_inputs(
        x,
        np.asarray(freqs_cos, np.float32),
        np.asarray(freqs_sin, np.float32),
        np.asarray(wq, np.float32),
        np.asarray(wk, np.float32),
        np.asarray(wv, np.float32),
        np.asarray(wo, np.float32),
    )
    res = run_bass_kernel_spmd(nc, in_maps, core_ids=list(range(N_CORES)), trace=_trace)
    if _result_box is not None:
        _result_box.append(res)
    out = np.concatenate([res.results[c]["out"] for c in range(N_CORES)], axis=1)
    return out.reshape(B, S, D).astype(np.float32)
